# revision 8
# baseline (speedup 1.0000x reference)
"""Trainium2 Bass kernel for a single-layer attention module (RMSNorm + QKV +
RoPE + causal attention over a KV cache + output projection), tensor-parallel
over 8 NeuronCores (4 heads each), per-head AllGather of attention outputs,
and per-core output-column blocks of the final projection.

Self-contained: takes FULL inputs, returns the FULL [1024, 4096] f32 output.
"""

import sys

sys.path.insert(0, "/opt/trn_rl_repo")

import numpy as np
import ml_dtypes

import concourse.bass as bass  # noqa: F401
import concourse.bacc as bacc
import concourse.tile as tile
from concourse import mybir
from concourse import bass_utils

BF16 = ml_dtypes.bfloat16
F32 = np.float32

N_CORES = 8
D, H, HD, S, C = 4096, 32, 128, 1024, 2048
T = C + S          # 3072 total keys
HL = H // N_CORES  # 4 heads per core
OC = HL * HD       # 512 local attention features per core
NDK = D // 128     # 32 contraction tiles over D
NTC = C // 128     # 16 cache t-tiles
NTN = S // 128     # 8 new-key t-tiles
EPS = 1e-6
THETA = 10000.0

bf = mybir.dt.bfloat16
f32 = mybir.dt.float32


def _build_nc():
    nc = bacc.Bacc("TRN2", target_bir_lowering=False, debug=False,
                   num_devices=N_CORES)

    # ---- DRAM I/O ----
    xs_t = nc.dram_tensor("xs_t", [128, NDK * S], bf, kind="ExternalInput")
    wq_col = nc.dram_tensor("wq_col", [HL, 128, NDK * 128], bf, kind="ExternalInput")
    wk_col = nc.dram_tensor("wk_col", [HL, 128, NDK * 128], bf, kind="ExternalInput")
    wv_col = nc.dram_tensor("wv_col", [HL, 128, NDK * 128], bf, kind="ExternalInput")
    wo_blk = nc.dram_tensor("wo_blk", [HL, 128, 8 * OC], bf, kind="ExternalInput")
    ckt = nc.dram_tensor("ckt", [HL, 128, C], bf, kind="ExternalInput")
    cvr = nc.dram_tensor("cvr", [HL, 128, C], bf, kind="ExternalInput")
    cosT = nc.dram_tensor("cosT", [128, S], bf, kind="ExternalInput")
    sinT = nc.dram_tensor("sinT", [128, S], bf, kind="ExternalInput")
    maskW = nc.dram_tensor("maskW", [128, 2 * S], bf, kind="ExternalInput")
    ones_d = nc.dram_tensor("ones_d", [128, 128], bf, kind="ExternalInput")
    id_d = nc.dram_tensor("id_d", [128, 128], bf, kind="ExternalInput")
    y = nc.dram_tensor("y", [S, OC], f32, kind="ExternalOutput")

    with tile.TileContext(nc) as tc:
        with (
            tc.tile_pool(name="const", bufs=1) as cpool,
            tc.tile_pool(name="qk", bufs=1) as qkpool,
            tc.tile_pool(name="kv", bufs=2) as kvpool,
            tc.tile_pool(name="exp", bufs=4) as epool,
            tc.tile_pool(name="att", bufs=1) as apool,
            tc.tile_pool(name="rec", bufs=2) as recpool,
            tc.tile_pool(name="dram", bufs=1, space="DRAM") as dpool,
        ):
            # ---- constants ----
            ones_t = cpool.tile([128, 128], bf, name="ones_t")
            nc.sync.dma_start(ones_t[:], ones_d[:])
            id_t = cpool.tile([128, 128], bf, name="id_t")
            nc.sync.dma_start(id_t[:], id_d[:])
            cos_t = cpool.tile([128, S], bf, name="cos_t")
            nc.sync.dma_start(cos_t[:], cosT[:])
            sin_t = cpool.tile([128, S], bf, name="sin_t")
            nc.sync.dma_start(sin_t[:], sinT[:])
            mask_t = cpool.tile([128, 2 * S], bf, name="mask_t")
            nc.sync.dma_start(mask_t[:], maskW[:])

            # persistent per-head results
            qr = qkpool.tile([128, HL * S], bf, name="qr")
            kr = qkpool.tile([128, HL * S], bf, name="kr")
            v_sb = qkpool.tile([128, HL * S], bf, name="v_sb")
            attnT = apool.tile([128, HL * S], bf, name="attnT")
            ag_in = [dpool.tile([128, S], bf, name=f"ag_in{h}") for h in range(HL)]
            ag_out = [dpool.tile([N_CORES * 128, S], bf, name=f"ag_out{h}",
                                 addr_space="Shared") for h in range(HL)]

            # =========== scope A: load xs, RMSNorm stats + projections ===========
            # Projections run on RAW xs^T (the 1/rms scale is applied after the
            # matmul, by linearity), so PE work starts as soon as DMA lands.
            with (
                tc.tile_pool(name="xs", bufs=8) as xpool,
                tc.tile_pool(name="sq", bufs=4) as sqpool,
                tc.tile_pool(name="nrm", bufs=1) as npool,
                tc.tile_pool(name="wcol", bufs=2) as wpool,
                tc.tile_pool(name="hh", bufs=2) as hpool,
                tc.tile_pool(name="rope", bufs=2) as rpool,
                tc.tile_pool(name="psA", bufs=2, space="PSUM") as psA,
                tc.tile_pool(name="psT", bufs=2, space="PSUM") as psT,
            ):
                xs_ch = []
                for g in range(8):
                    xc = xpool.tile([128, 4 * S], bf, name="xs_ch")
                    nc.sync.dma_start(xc[:], xs_t[:, g * 4 * S:(g + 1) * 4 * S])
                    xs_ch.append(xc)

                def xs_v(dk):  # [128, S] view of raw xs^T d-tile dk
                    return xs_ch[dk // 4][:, (dk % 4) * S:(dk % 4 + 1) * S]

                ps_ss = psA.tile([128, S], f32, name="psp")
                rsq_box = {}

                def emit_norm_dk(dk):
                    sqt = sqpool.tile([128, S], bf, name="sqt")
                    nc.vector.tensor_mul(sqt[:], xs_v(dk), xs_v(dk))
                    for sc in range(2):
                        nc.tensor.matmul(
                            ps_ss[:, sc * 512:(sc + 1) * 512],
                            ones_t[:], sqt[:, sc * 512:(sc + 1) * 512],
                            start=(dk == 0), stop=(dk == NDK - 1))

                def emit_rsq():
                    ssum = npool.tile([128, S], f32, name="ssum")
                    nc.scalar.activation(ssum[:], ps_ss[:],
                                         mybir.ActivationFunctionType.Copy,
                                         bias=EPS, scale=1.0 / D)
                    rcp = npool.tile([128, S], f32, name="rcp")
                    nc.vector.reciprocal(rcp[:], ssum[:])
                    rsq = npool.tile([128, S], bf, name="rsq")
                    nc.scalar.sqrt(rsq[:], rcp[:])
                    rsq_box["rsq"] = rsq

                def rope(dst, src):
                    # dst = src * cos2 + rot(src) * sin2,
                    # rot(src) = [src_hi; src_lo] via SBUF->SBUF DMA
                    rot = rpool.tile([128, S], bf, name="rot")
                    nc.sync.dma_start(rot[0:64, :], src[64:128, :])
                    nc.sync.dma_start(rot[64:128, :], src[0:64, :])
                    ta = rpool.tile([128, S], bf, name="ta")
                    nc.vector.tensor_mul(ta[:], src[:], cos_t[:])
                    tb = rpool.tile([128, S], bf, name="tb")
                    nc.vector.tensor_mul(tb[:], rot[:], sin_t[:])
                    nc.vector.tensor_add(dst[:], ta[:], tb[:])

                first = True
                for which, wsrc in (("q", wq_col), ("k", wk_col), ("v", wv_col)):
                    for h in range(HL):
                        wcol = wpool.tile([128, NDK * 128], bf, name="wcol")
                        nc.sync.dma_start(wcol[:], wsrc[h])
                        psp = psA.tile([128, S], f32, name="psp")
                        for dk in range(NDK):
                            if first:
                                emit_norm_dk(dk)
                            for sc in range(2):
                                nc.tensor.matmul(
                                    psp[:, sc * 512:(sc + 1) * 512],
                                    wcol[:, dk * 128:(dk + 1) * 128],
                                    xs_v(dk)[:, sc * 512:(sc + 1) * 512],
                                    start=(dk == 0), stop=(dk == NDK - 1))
                        if first:
                            emit_rsq()
                            first = False
                        hh = hpool.tile([128, S], bf, name="hh")
                        nc.scalar.copy(hh[:], psp[:])
                        hh2 = hpool.tile([128, S], bf, name="hh2")
                        nc.vector.tensor_mul(hh2[:], hh[:], rsq_box["rsq"][:])
                        if which == "q":
                            rope(qr[:, h * S:(h + 1) * S], hh2[:])
                        elif which == "k":
                            rope(kr[:, h * S:(h + 1) * S], hh2[:])
                        else:
                            for tj in range(NTN):
                                ptr = psT.tile([128, 128], bf, name="ptr")
                                nc.tensor.transpose(
                                    ptr[:], hh2[:, tj * 128:(tj + 1) * 128], id_t[:])
                                nc.scalar.copy(
                                    v_sb[:, h * S + tj * 128: h * S + (tj + 1) * 128],
                                    ptr[:])

            # =========== scope B: attention (software-pipelined) ===========
            with (
                tc.tile_pool(name="psS", bufs=2, space="PSUM") as psS,
                tc.tile_pool(name="psDen", bufs=2, space="PSUM") as psDen,
                tc.tile_pool(name="psO", bufs=4, space="PSUM") as psO,
            ):
                for h in range(HL):
                    ck_sb = kvpool.tile([128, C], bf, name="ck_sb")
                    nc.sync.dma_start(ck_sb[:], ckt[h])
                    cv_sb = kvpool.tile([128, C], bf, name="cv_sb")
                    nc.sync.dma_start(cv_sb[:], cvr[h])
                    qh = qr[:, h * S:(h + 1) * S]
                    den = [psDen.tile([128, 512], f32, name="den") for _ in range(2)]
                    Oc = [psO.tile([128, 512], f32, name="Oc") for _ in range(2)]

                    def lts_ltv(ti):
                        if ti < NTC:
                            return (ck_sb[:, ti * 128:(ti + 1) * 128],
                                    cv_sb[:, ti * 128:(ti + 1) * 128])
                        tn = ti - NTC
                        return (kr[:, h * S + tn * 128: h * S + (tn + 1) * 128],
                                v_sb[:, h * S + tn * 128: h * S + (tn + 1) * 128])

                    groups = []
                    for ti in range(NTC + NTN):
                        for sc in range(2):
                            if ti >= NTC and (ti - NTC) * 128 > sc * 512 + 511:
                                continue       # fully masked tile
                            groups.append((ti, sc))
                    glast = {sc: max(g for g in groups if g[1] == sc)
                             for sc in range(2)}

                    def emit_epilogue(sc):
                        rec = recpool.tile([128, 512], f32, name="rec")
                        nc.vector.reciprocal(rec[:], den[sc][:])
                        nc.vector.tensor_mul(
                            attnT[:, h * S + sc * 512: h * S + (sc + 1) * 512],
                            Oc[sc][:], rec[:])
                        nc.sync.dma_start(
                            ag_in[h][:, sc * 512:(sc + 1) * 512],
                            attnT[:, h * S + sc * 512: h * S + (sc + 1) * 512])

                    def emit_den_O(g, e_t):
                        ti, sc = g
                        _, lv = lts_ltv(ti)
                        nc.tensor.matmul(den[sc][:], ones_t[:], e_t[:],
                                         start=(ti == 0), stop=(g == glast[sc]))
                        nc.tensor.matmul(Oc[sc][:], lv, e_t[:],
                                         start=(ti == 0), stop=(g == glast[sc]))
                        if g == glast[sc]:
                            emit_epilogue(sc)

                    pend = None  # (group, e_tile) one step behind
                    for g in groups:
                        ti, sc = g
                        ls, _ = lts_ltv(ti)
                        ps = psS.tile([128, 512], f32, name="ps")
                        nc.tensor.matmul(
                            ps[:], ls, qh[:, sc * 512:(sc + 1) * 512],
                            start=True, stop=True)
                        e = epool.tile([128, 512], bf, name="e")
                        nc.scalar.activation(
                            e[:], ps[:], mybir.ActivationFunctionType.Exp)
                        if ti >= NTC and (ti - NTC) * 128 + 127 > sc * 512:
                            off = S - (ti - NTC) * 128 + sc * 512
                            nc.vector.tensor_mul(
                                e[:], e[:], mask_t[:, off:off + 512])
                        if pend is not None:
                            emit_den_O(*pend)
                        pend = (g, e)
                    emit_den_O(*pend)

                    # per-head AllGather right after this head's output DMA
                    nc.gpsimd.collective_compute(
                        "AllGather", mybir.AluOpType.bypass,
                        replica_groups=[list(range(N_CORES))],
                        ins=[ag_in[h][:]], outs=[ag_out[h][:]])

            # =========== scope C: output projection ===========
            # o-tile (= global head g = 4r + h_local) rows live in ag_out[h][r].
            with (
                tc.tile_pool(name="psY", bufs=8, space="PSUM") as psY,
                tc.tile_pool(name="ag", bufs=2) as agpool,
                tc.tile_pool(name="wo", bufs=2) as wopool,
                tc.tile_pool(name="yout", bufs=2) as ypool,
            ):
                ps_y = [psY.tile([128, OC], f32, name="ps_y") for _ in range(8)]
                n_ot = 0
                for h in range(HL):
                    ag_sb = agpool.tile([128, 8 * S], bf, name="ag_sb")
                    nc.sync.dma_start(
                        ag_sb[:].rearrange("p (r s) -> p r s", r=8),
                        ag_out[h][:].rearrange("(r p) s -> p r s", p=128))
                    wo_sb = wopool.tile([128, 8 * OC], bf, name="wo_sb")
                    nc.sync.dma_start(wo_sb[:], wo_blk[h])
                    for r in range(8):
                        for sc8 in range(8):
                            nc.tensor.matmul(
                                ps_y[sc8][:],
                                ag_sb[:, r * S + sc8 * 128: r * S + (sc8 + 1) * 128],
                                wo_sb[:, r * OC:(r + 1) * OC],
                                start=(n_ot == 0), stop=(n_ot == 31))
                        n_ot += 1
                for sc8 in range(8):
                    ysb = ypool.tile([128, OC], f32, name="ysb")
                    nc.scalar.copy(ysb[:], ps_y[sc8][:])
                    nc.sync.dma_start(y[sc8 * 128:(sc8 + 1) * 128, :], ysb[:])

    nc.compile()
    return nc


def _host_prep(xs, cache_k, cache_v, norm_w, wq, wk, wv, wo):
    """Build the 8 per-core input maps (all layout work done on host)."""
    xs = np.asarray(xs, F32)
    cache_k = np.asarray(cache_k, F32)
    cache_v = np.asarray(cache_v, F32)
    norm_w = np.asarray(norm_w, F32)
    wq, wk, wv, wo = (np.asarray(w, F32) for w in (wq, wk, wv, wo))

    # xs^T tiled: [128, dk*S]
    xs_t = np.ascontiguousarray(
        xs.T.reshape(NDK, 128, S).transpose(1, 0, 2).reshape(128, NDK * S)
    ).astype(BF16)

    # RoPE tables (positions C..C+S-1), transposed [freq, s]
    half = HD // 2
    inv_freq = 1.0 / (THETA ** (np.arange(0, half, dtype=np.float64) * 2.0 / HD))
    pos = np.arange(S, dtype=np.float64) + C
    ang = np.outer(pos, inv_freq)          # [S, 64]
    cos1 = np.cos(ang).T.astype(F32)       # [64, S]
    sin1 = np.sin(ang).T.astype(F32)
    cosT = np.vstack([cos1, cos1]).astype(BF16)          # [128, S]
    sinT = np.vstack([-sin1, sin1]).astype(BF16)         # rotate-half signs

    # sliding causal mask window: W[p, j] = 1 iff j >= S + p
    jj = np.arange(2 * S)[None, :]
    pp = np.arange(128)[:, None]
    maskW = (jj >= S + pp).astype(F32).astype(BF16)

    ones_d = np.ones((128, 128), F32).astype(BF16)
    id_d = np.eye(128, dtype=F32).astype(BF16)

    sc_q = F32(1.0) / np.sqrt(F32(HD))

    in_maps = []
    for c in range(N_CORES):
        osl = slice(OC * c, OC * (c + 1))
        hsl = slice(HL * c, HL * (c + 1))
        # fold norm_w into wq/wk/wv; fold 1/sqrt(HD) into wq
        wq_c = (wq[osl] * norm_w[None, :]) * sc_q   # [512, 4096]
        wk_c = wk[osl] * norm_w[None, :]
        wv_c = wv[osl] * norm_w[None, :]

        def col_layout(w_c):
            # [HL, 128, NDK*128]: [h, p, dk*128 + j] = w_c[h*128+j, dk*128+p]
            m = w_c.reshape(HL, 128, NDK, 128)          # [h, j, dk, p]
            return np.ascontiguousarray(
                m.transpose(0, 3, 2, 1).reshape(HL, 128, NDK * 128)).astype(BF16)

        wq_col = col_layout(wq_c)
        wk_col = col_layout(wk_c)
        wv_col = col_layout(wv_c)

        # wo block: [h, p, r*OC + j] = wo[OC*c + j, (4r + h)*128 + p]
        wo_c = wo[osl]                                  # [512, 4096]
        m = wo_c.reshape(OC, 8, HL, 128)                # [j, r, h, p]
        wo_blk = np.ascontiguousarray(
            m.transpose(2, 3, 1, 0).reshape(HL, 128, 8 * OC)).astype(BF16)

        # cache K^T per head: [h, p(hd), t]
        ck = np.ascontiguousarray(
            cache_k[:, hsl, :].transpose(1, 2, 0)).astype(BF16)   # [HL, 128, C]
        # cache V tiles: [h, p(t%128), ti*128 + hd]
        cv = np.ascontiguousarray(
            cache_v[:, hsl, :].reshape(NTC, 128, HL, HD)
            .transpose(2, 1, 0, 3).reshape(HL, 128, C)).astype(BF16)

        in_maps.append({
            "xs_t": xs_t, "wq_col": wq_col, "wk_col": wk_col, "wv_col": wv_col,
            "wo_blk": wo_blk, "ckt": ck, "cvr": cv,
            "cosT": cosT, "sinT": sinT, "maskW": maskW,
            "ones_d": ones_d, "id_d": id_d,
        })
    return in_maps


_NC_CACHE = {}


def kernel(xs, cache_k, cache_v, norm_w, wq, wk, wv, wo, _trace=False):
    if "nc" not in _NC_CACHE:
        _NC_CACHE["nc"] = _build_nc()
    nc = _NC_CACHE["nc"]
    in_maps = _host_prep(xs, cache_k, cache_v, norm_w, wq, wk, wv, wo)
    res = bass_utils.run_bass_kernel_spmd(
        nc, in_maps, core_ids=list(range(N_CORES)), trace=_trace)
    out = np.concatenate([res.results[c]["y"] for c in range(N_CORES)], axis=1)
    if _trace:
        kernel.last_exec_time_ns = res.exec_time_ns
        kernel.last_results = res
    return out
